# revision 52
# baseline (speedup 1.0000x reference)
"""MetaGAT message-passing kernel for Trainium2 (8 NeuronCores, Bass/Tile).

Strategy (node-sharded, fully local segment softmax, bf16 datapath):
  * dst has exactly K=8 incoming edges per node; edges are grouped by dst on
    the host and each core gets 63 nodes (500 padded to 504) with all their
    incoming edges.  Segment max/sum never crosses cores - no collectives.
  * The per-edge matmul inputs [s_src(e); s_dst(e)] (K=2F=128) are
    PRE-ASSEMBLED on the host into a bf16 tensor laid out exactly as the PE
    consumes them (one contiguous [128, 8*192] tile per node), so the device
    does plain sequential DMA - no indirect gathers, no SBUF reshuffles, and
    every alpha matmul is a single non-accumulating K=128 x M=64 x N=192
    bf16 op (start=stop=True; PE tile positions never mix inside a group).
  * A second host tensor holds the src states in the "quadrant" layout that
    matches the alpha PSUM tiles, for the exp()*s_src product.
  * The hypernetwork weights w_e = h0*W3_0 + h1*W3_1 + B3 for all 504 edges
    are built on the PE with 64 tiny K=3 matmuls (one per output column f):
    lhsT = [W3_0[:,f]; W3_1[:,f]; B3[:,f]] (3x128), rhs = [h0; h1; 1]
    (3x504).  This replaces ~260us of DVE broadcast work.
  * exp() without max-subtraction (|alpha| small for this data), products
    and pairwise sums in bf16, 128->64 fold + pair-sum via accumulating
    matmuls (uniform tile position), reciprocal via the fast DVE
    approximation with the sigmoid(gate) scale folded into the den fold
    weights, output written in [f, bt] layout (no PE transpose) and
    re-transposed on the host.
"""

import numpy as np
import ml_dtypes

import concourse.bacc as bacc
import concourse.bass as bass
import concourse.mybir as mybir
import concourse.tile as tile
from concourse.bass_utils import run_bass_kernel_spmd

N, E, KE, B, T, F = 500, 4000, 8, 16, 12, 64
BT = B * T                  # 192
NCORES = 8
NPC = 63                    # nodes per core (8*63 = 504 >= 500, tail padded)
EPC = KE * NPC              # 504 edges per core
H1, H2 = 16, 2
f32 = mybir.dt.float32
bf16 = mybir.dt.bfloat16
AF = mybir.ActivationFunctionType
ALU = mybir.AluOpType

TRACE = False               # set True (module-level) to profile; see LAST_RESULTS
LAST_RESULTS = None

_cache = {}


def _build_program():
    nc = bacc.Bacc("TRN2", target_bir_lowering=False)

    # per node, 12*BT cols: 0:8*BT alpha-matmul rhs blocks (src;dst per
    # edge), 8*BT:12*BT src states in alpha-PSUM quadrant layout.  Nodes are
    # side by side on columns so one DMA fetches several nodes.
    rsp_d = nc.dram_tensor("rsp", [128, NPC * 12 * BT], bf16, kind="ExternalInput")
    featT_d = nc.dram_tensor("featT", [128, EPC], bf16, kind="ExternalInput")
    distR_d = nc.dram_tensor("distR", [1, EPC], bf16, kind="ExternalInput")
    w1ab_d = nc.dram_tensor("w1ab", [128, H1], bf16, kind="ExternalInput")
    w1c_d = nc.dram_tensor("w1c", [1, H1], bf16, kind="ExternalInput")
    b1_d = nc.dram_tensor("b1", [H1, 1], f32, kind="ExternalInput")
    w2_d = nc.dram_tensor("w2", [H1, H2], bf16, kind="ExternalInput")
    b2_d = nc.dram_tensor("b2", [H2, 1], f32, kind="ExternalInput")
    w3p_d = nc.dram_tensor("w3p", [3, F * 128], bf16, kind="ExternalInput")
    gate_d = nc.dram_tensor("gate", [1, 1], f32, kind="ExternalInput")
    foldm_d = nc.dram_tensor("foldm", [128, 64], bf16, kind="ExternalInput")
    out_d = nc.dram_tensor("out", [64, NPC * BT], bf16, kind="ExternalOutput")

    with tile.TileContext(nc) as tc:
        with tc.tile_pool(name="const", bufs=1) as cp:
            foldm = cp.tile([128, 64], bf16)
            nc.sync.dma_start(out=foldm[:], in_=foldm_d[:])
            ones1 = cp.tile([1, 128], f32)
            nc.vector.memset(ones1[:], 1.0)
            w3p = cp.tile([3, F * 128], bf16)
            nc.sync.dma_start(out=w3p[:], in_=w3p_d[:])
            featA = cp.tile([64, EPC], bf16)
            nc.sync.dma_start(out=featA[:], in_=featT_d[0:64, :])
            featB = cp.tile([64, EPC], bf16)
            nc.sync.dma_start(out=featB[:], in_=featT_d[64:128, :])
            distR = cp.tile([1, EPC], bf16)
            nc.sync.dma_start(out=distR[:], in_=distR_d[:])
            w1a = cp.tile([64, H1], bf16)
            nc.sync.dma_start(out=w1a[:], in_=w1ab_d[0:64, :])
            w1b = cp.tile([64, H1], bf16)
            nc.sync.dma_start(out=w1b[:], in_=w1ab_d[64:128, :])
            w1c = cp.tile([1, H1], bf16)
            nc.sync.dma_start(out=w1c[:], in_=w1c_d[:])
            b1t = cp.tile([H1, 1], f32)
            nc.sync.dma_start(out=b1t[:], in_=b1_d[:])
            w2t = cp.tile([H1, H2], bf16)
            nc.sync.dma_start(out=w2t[:], in_=w2_d[:])
            b2t = cp.tile([H2, 1], f32)
            nc.sync.dma_start(out=b2t[:], in_=b2_d[:])

            h3 = cp.tile([3, EPC], bf16)
            foldg = cp.tile([128, 64], bf16)
            # w_sb[:, 504*f + e] = w_e[k, f]  (k on partitions, f-major cols:
            # the w-build copies stay contiguous; alpha lhsT reads stride EPC)
            w_sb = cp.tile([128, F * EPC], bf16)
            wv = w_sb[:].rearrange("p (f e) -> p e f", e=EPC)

            # ---------- per-edge hypernetwork MLP ----------
            with tc.tile_pool(name="pre", bufs=1) as pp, \
                 tc.tile_pool(name="preps", bufs=1, space="PSUM") as pps:
                hp1 = pps.tile([H1, EPC], f32, tag="hp1", space="PSUM")
                nc.tensor.matmul(out=hp1[:], lhsT=w1a[:], rhs=featA[:],
                                 start=True, stop=False)
                nc.tensor.matmul(out=hp1[:], lhsT=w1b[:], rhs=featB[:],
                                 start=False, stop=False)
                nc.tensor.matmul(out=hp1[:], lhsT=w1c[:], rhs=distR[:],
                                 start=False, stop=True)
                h1sb = pp.tile([H1, EPC], bf16, tag="h1sb")
                nc.scalar.activation(out=h1sb[:], in_=hp1[:], func=AF.Sigmoid,
                                     bias=b1t[:, 0:1])
                hp2 = pps.tile([H2, EPC], f32, tag="hp2", space="PSUM")
                nc.tensor.matmul(out=hp2[:], lhsT=w2t[:], rhs=h1sb[:],
                                 start=True, stop=True)
                nc.vector.memset(h3[:], 1.0)
                nc.scalar.activation(out=h3[0:2, :], in_=hp2[:], func=AF.Sigmoid,
                                     bias=b2t[:, 0:1])

                # foldg = foldm / sigmoid(gate): den fold absorbs the gate so
                # out = max(num,0) * (1/den') with den' = den/sig(gate)
                gt = pp.tile([1, 1], f32, tag="gt")
                nc.sync.dma_start(out=gt[:], in_=gate_d[:])
                gs = pp.tile([1, 1], f32, tag="gs")
                nc.scalar.activation(out=gs[:], in_=gt[:], func=AF.Sigmoid)
                gsr = pp.tile([1, 1], f32, tag="gsr")
                nc.vector.reciprocal(out=gsr[:], in_=gs[:])
                gp = pps.tile([128, 1], f32, tag="gp", space="PSUM")
                nc.tensor.matmul(out=gp[:], lhsT=ones1[:], rhs=gsr[:],
                                 start=True, stop=True)
                grb = pp.tile([128, 1], f32, tag="grb")
                nc.scalar.copy(out=grb[:], in_=gp[:])
                nc.vector.tensor_scalar_mul(foldg[:], foldm[:], grb[:, 0:1])

            # ---------- w build: one K=3 matmul per feature column f ----------
            with tc.tile_pool(name="wps", bufs=6, space="PSUM") as wps_p:
                for f in range(F):
                    wps = wps_p.tile([128, EPC], f32, tag="wps", space="PSUM")
                    nc.tensor.matmul(out=wps[:], lhsT=w3p[:, 128 * f:128 * (f + 1)],
                                     rhs=h3[:], start=True, stop=True)
                    dest = w_sb[:, EPC * f:EPC * (f + 1)]
                    if f % 2 == 0:
                        nc.vector.tensor_scalar_mul(dest, wps[:], 1.0)
                    else:
                        nc.scalar.copy(out=dest, in_=wps[:])

            # ---------- main per-node loop ----------
            outbuf = cp.tile([64, NPC * BT], bf16)
            NCHUNK = 3                          # nodes fetched per DMA
            with tc.tile_pool(name="rh", bufs=4) as rh_p, \
                 tc.tile_pool(name="lrl", bufs=6) as lrl_p, \
                 tc.tile_pool(name="ex", bufs=6) as ex_p, \
                 tc.tile_pool(name="pr", bufs=6) as pr_p, \
                 tc.tile_pool(name="fin", bufs=6) as fin_p, \
                 tc.tile_pool(name="aps", bufs=5, space="PSUM") as aps_p, \
                 tc.tile_pool(name="fps", bufs=3, space="PSUM") as fps_p:
                for j in range(NPC):
                    jc = j % NCHUNK
                    if jc == 0:
                        rspc = rh_p.tile([128, NCHUNK * 12 * BT], bf16, tag="rsp")
                        nc.sync.dma_start(
                            out=rspc[:],
                            in_=rsp_d[:, 12 * BT * j:12 * BT * (j + NCHUNK)])
                    rsp = rspc[:, 12 * BT * jc:12 * BT * (jc + 1)]
                    spk2 = rsp[:, 8 * BT:12 * BT]

                    lrl = lrl_p.tile([128, 4 * BT], bf16, tag="lrl")
                    aps = []
                    for hh in range(2):
                        eb = 8 * j + 4 * hh
                        c0 = 4 * hh * BT
                        ap = aps_p.tile([128, 2 * BT], f32, tag="ap", space="PSUM")
                        for q in range(4):
                            e = eb + q
                            rq = (q % 2) * 64              # psum row base
                            cq = (q // 2) * BT             # psum col base
                            nc.tensor.matmul(
                                out=ap[rq:rq + 64, cq:cq + BT],
                                lhsT=wv[:, e:e + 1, :],
                                rhs=rsp[:, c0 + q * BT:c0 + (q + 1) * BT],
                                start=True, stop=True)
                        aps.append(ap)

                        half = lrl[:, 2 * BT * hh:2 * BT * (hh + 1)]
                        if hh == 0:
                            # lrl = alpha - 0.99*min(alpha, 0) == leaky_relu
                            lt = lrl_p.tile([128, 2 * BT], f32, tag="lt")
                            nc.vector.tensor_scalar(
                                out=lt[:], in0=ap[:], scalar1=0.0, scalar2=-0.99,
                                op0=ALU.min, op1=ALU.mult)
                            nc.vector.tensor_add(out=half, in0=lt[:], in1=ap[:])
                        else:
                            nc.scalar.activation(out=half, in_=ap[:],
                                                 func=AF.Prelu, alpha=0.01)

                    ext = ex_p.tile([128, 4 * BT], bf16, tag="ext")
                    nc.scalar.activation(out=ext[:], in_=lrl[:], func=AF.Exp)
                    prt = pr_p.tile([128, 4 * BT], bf16, tag="prt")
                    nc.gpsimd.tensor_mul(out=prt[:], in0=ext[:], in1=spk2)

                    fold = fps_p.tile([64, 2 * BT], f32, tag="fold", space="PSUM")
                    for q in range(4):
                        nc.tensor.matmul(out=fold[:, 0:BT], lhsT=foldg[:],
                                         rhs=ext[:, q * BT:(q + 1) * BT],
                                         start=(q == 0), stop=(q == 3))
                    for q in range(4):
                        nc.tensor.matmul(out=fold[:, BT:2 * BT], lhsT=foldm[:],
                                         rhs=prt[:, q * BT:(q + 1) * BT],
                                         start=(q == 0), stop=(q == 3))

                    rden = fin_p.tile([64, BT], f32, tag="rden")
                    nc.vector.reciprocal_approx_fast(out=rden[:], in_=fold[:, 0:BT])
                    nc.vector.scalar_tensor_tensor(
                        out=outbuf[:, BT * j:BT * (j + 1)],
                        in0=fold[:, BT:2 * BT], scalar=0.0,
                        in1=rden[:], op0=ALU.max, op1=ALU.mult)
                nc.sync.dma_start(out=out_d[:], in_=outbuf[:])

    nc.compile()
    return nc


def _host_prep(state, feature, dist, src, dst, w1, b1, w2, b2, w3, b3, gate_weight):
    # sT[n, f, bt] in bf16 for pre-gathered src/dst state tiles
    sT = np.ascontiguousarray(
        state.transpose(2, 3, 0, 1).reshape(N, F, BT)).astype(ml_dtypes.bfloat16)
    sT_pad = np.zeros((NCORES * NPC, F, BT), ml_dtypes.bfloat16)
    sT_pad[:N] = sT

    # group edges by destination: edges_by[n, k] = id of n's k-th incoming edge
    order = np.argsort(dst, kind="stable")
    edges_by = order.reshape(N, KE)            # [N, KE]

    W3 = w3.reshape(2, 2 * F, F)
    # w3p[p, 128*f + k] = [W3_0 | W3_1 | B3][p][k, f]
    w3p = np.stack([W3[0], W3[1], b3.reshape(2 * F, F)], axis=0)  # [3, 128, 64]
    w3p = np.ascontiguousarray(w3p.transpose(0, 2, 1).reshape(3, F * 128)
                               ).astype(ml_dtypes.bfloat16)

    eye = np.eye(64, dtype=np.float32)
    base = {
        "w3p": w3p,
        "foldm": np.ascontiguousarray(np.concatenate([eye, eye], axis=0)
                                      ).astype(ml_dtypes.bfloat16),
        "w1ab": np.ascontiguousarray(w1[0:128]).astype(ml_dtypes.bfloat16),
        "w1c": np.ascontiguousarray(w1[128:129]).astype(ml_dtypes.bfloat16),
        "b1": np.ascontiguousarray(b1.reshape(H1, 1), np.float32),
        "w2": np.ascontiguousarray(w2).astype(ml_dtypes.bfloat16),
        "b2": np.ascontiguousarray(b2.reshape(H2, 1), np.float32),
        "gate": np.ascontiguousarray(gate_weight.reshape(1, 1), np.float32),
    }

    in_maps = []
    for c in range(NCORES):
        nodes = np.arange(c * NPC, (c + 1) * NPC)
        valid = nodes < N
        nodes_c = np.where(valid, nodes, 0)
        eids = edges_by[nodes_c]               # [NPC, KE] node-major edges
        src_c = np.where(valid[:, None], src[eids], 0).astype(np.int64)
        dist_c = np.where(valid[:, None], dist[eids, 0], 0.0).astype(np.float32)

        g = sT[src_c]                          # [NPC, KE, F, BT] bf16
        dn = sT_pad[nodes]                     # [NPC, F, BT] bf16

        # rsp: per node [128, 12*192].  Cols 0:8BT: col-block k rows 0:64 =
        # sT[src(e_k)], rows 64:128 = sT[dst node].  Cols 8BT:12BT: col-block
        # q rows 0:64 = sT[src(e_2q)], rows 64:128 = sT[src(e_2q+1)] (the
        # alpha-PSUM quadrant layout, for the exp*s_src product).
        gt = g.transpose(0, 2, 1, 3)           # [NPC, F, KE, BT]
        # [128 partitions, NPC nodes, 12 col-blocks, BT]
        rsp = np.empty((2, F, NPC, 12, BT), ml_dtypes.bfloat16)
        rsp[0, :, :, 0:8] = gt.transpose(1, 0, 2, 3)
        rsp[1, :, :, 0:8] = dn.transpose(1, 0, 2)[:, :, None, :]
        rsp[0, :, :, 8:12] = gt.transpose(1, 0, 2, 3)[:, :, 0::2]
        rsp[1, :, :, 8:12] = gt.transpose(1, 0, 2, 3)[:, :, 1::2]
        rsp = np.ascontiguousarray(rsp.reshape(128, NPC * 12 * BT))

        # featT: [128, EPC] = [feature[src].T ; feature[dst].T], node-major edges
        fsrc = np.where(valid[:, None, None], feature[src_c], 0.0)   # [NPC,KE,F]
        fdst = np.broadcast_to(
            np.where(valid[:, None], feature[nodes_c], 0.0)[:, None, :],
            (NPC, KE, F))
        featT = np.concatenate([
            fsrc.reshape(EPC, F).T, fdst.reshape(EPC, F).T], axis=0)

        m = dict(base)
        m.update({
            "rsp": rsp,
            "featT": np.ascontiguousarray(featT).astype(ml_dtypes.bfloat16),
            "distR": np.ascontiguousarray(dist_c.reshape(1, EPC)).astype(ml_dtypes.bfloat16),
        })
        in_maps.append(m)
    return in_maps


def kernel(state, feature, dist, w1, b1, w2, b2, w3, b3, gate_weight, src, dst):
    global LAST_RESULTS
    state = np.asarray(state, np.float32)
    if "nc" not in _cache:
        _cache["nc"] = _build_program()
    nc = _cache["nc"]

    in_maps = _host_prep(np.asarray(state), np.asarray(feature, np.float32),
                         np.asarray(dist, np.float32),
                         np.asarray(src), np.asarray(dst),
                         np.asarray(w1, np.float32), np.asarray(b1, np.float32),
                         np.asarray(w2, np.float32), np.asarray(b2, np.float32),
                         np.asarray(w3, np.float32), np.asarray(b3, np.float32),
                         np.asarray(gate_weight, np.float32))

    res = run_bass_kernel_spmd(nc, in_maps, core_ids=list(range(NCORES)), trace=TRACE)
    LAST_RESULTS = res

    # out_c: [64, NPC*BT] bf16 -> [F, NPC, B, T] -> [B, T, NPC, F]
    parts = [np.asarray(res.results[c]["out"]).astype(np.float32)
             .reshape(F, NPC, B, T).transpose(2, 3, 1, 0) for c in range(NCORES)]
    full = np.concatenate(parts, axis=2)[:, :, :N, :]
    return np.ascontiguousarray(full)


# revision 53
# speedup vs baseline: 1.1285x; 1.1285x over previous
"""MetaGAT message-passing kernel for Trainium2 (8 NeuronCores, Bass/Tile).

Strategy (node-sharded, fully local segment softmax, bf16 datapath):
  * dst has exactly K=8 incoming edges per node; edges are grouped by dst on
    the host and each core gets 63 nodes (500 padded to 504) with all their
    incoming edges.  Segment max/sum never crosses cores - no collectives.
  * The per-edge matmul inputs [s_src(e); s_dst(e)] (K=2F=128) are
    PRE-ASSEMBLED on the host into a bf16 tensor laid out exactly as the PE
    consumes them (one contiguous [128, 8*192] tile per node), so the device
    does plain sequential DMA - no indirect gathers, no SBUF reshuffles, and
    every alpha matmul is a single non-accumulating K=128 x M=64 x N=192
    bf16 op (start=stop=True; PE tile positions never mix inside a group).
  * A second host tensor holds the src states in the "quadrant" layout that
    matches the alpha PSUM tiles, for the exp()*s_src product.
  * The hypernetwork weights w_e = h0*W3_0 + h1*W3_1 + B3 for all 504 edges
    are built on the PE with 64 tiny K=3 matmuls (one per output column f):
    lhsT = [W3_0[:,f]; W3_1[:,f]; B3[:,f]] (3x128), rhs = [h0; h1; 1]
    (3x504).  This replaces ~260us of DVE broadcast work.
  * exp() without max-subtraction (|alpha| small for this data), products
    and pairwise sums in bf16, 128->64 fold + pair-sum via accumulating
    matmuls (uniform tile position), reciprocal via the fast DVE
    approximation with the sigmoid(gate) scale folded into the den fold
    weights, output written in [f, bt] layout (no PE transpose) and
    re-transposed on the host.
"""

import numpy as np
import ml_dtypes

import concourse.bacc as bacc
import concourse.bass as bass
import concourse.mybir as mybir
import concourse.tile as tile
from concourse.bass_utils import run_bass_kernel_spmd

N, E, KE, B, T, F = 500, 4000, 8, 16, 12, 64
BT = B * T                  # 192
NCORES = 8
NPC = 63                    # nodes per core (8*63 = 504 >= 500, tail padded)
EPC = KE * NPC              # 504 edges per core
H1, H2 = 16, 2
f32 = mybir.dt.float32
bf16 = mybir.dt.bfloat16
AF = mybir.ActivationFunctionType
ALU = mybir.AluOpType

TRACE = False               # set True (module-level) to profile; see LAST_RESULTS
LAST_RESULTS = None

_cache = {}


def _build_program():
    nc = bacc.Bacc("TRN2", target_bir_lowering=False)

    # per node, 12*BT cols: 0:8*BT alpha-matmul rhs blocks (src;dst per
    # edge), 8*BT:12*BT src states in alpha-PSUM quadrant layout.  Nodes are
    # side by side on columns so one DMA fetches several nodes.
    rsp_d = nc.dram_tensor("rsp", [128, NPC * 12 * BT], bf16, kind="ExternalInput")
    featT_d = nc.dram_tensor("featT", [128, EPC], bf16, kind="ExternalInput")
    distR_d = nc.dram_tensor("distR", [1, EPC], bf16, kind="ExternalInput")
    w1ab_d = nc.dram_tensor("w1ab", [128, H1], bf16, kind="ExternalInput")
    w1c_d = nc.dram_tensor("w1c", [1, H1], bf16, kind="ExternalInput")
    b1_d = nc.dram_tensor("b1", [H1, 1], f32, kind="ExternalInput")
    w2_d = nc.dram_tensor("w2", [H1, H2], bf16, kind="ExternalInput")
    b2_d = nc.dram_tensor("b2", [H2, 1], f32, kind="ExternalInput")
    w3p_d = nc.dram_tensor("w3p", [3, F * 128], bf16, kind="ExternalInput")
    gate_d = nc.dram_tensor("gate", [1, 1], f32, kind="ExternalInput")
    foldm_d = nc.dram_tensor("foldm", [128, 64], bf16, kind="ExternalInput")
    out_d = nc.dram_tensor("out", [64, NPC * BT], bf16, kind="ExternalOutput")

    with tile.TileContext(nc) as tc:
        with tc.tile_pool(name="const", bufs=1) as cp:
            foldm = cp.tile([128, 64], bf16)
            nc.sync.dma_start(out=foldm[:], in_=foldm_d[:])
            ones1 = cp.tile([1, 128], f32)
            nc.vector.memset(ones1[:], 1.0)
            w3p = cp.tile([3, F * 128], bf16)
            nc.sync.dma_start(out=w3p[:], in_=w3p_d[:])
            featA = cp.tile([64, EPC], bf16)
            nc.sync.dma_start(out=featA[:], in_=featT_d[0:64, :])
            featB = cp.tile([64, EPC], bf16)
            nc.sync.dma_start(out=featB[:], in_=featT_d[64:128, :])
            distR = cp.tile([1, EPC], bf16)
            nc.sync.dma_start(out=distR[:], in_=distR_d[:])
            w1a = cp.tile([64, H1], bf16)
            nc.sync.dma_start(out=w1a[:], in_=w1ab_d[0:64, :])
            w1b = cp.tile([64, H1], bf16)
            nc.sync.dma_start(out=w1b[:], in_=w1ab_d[64:128, :])
            w1c = cp.tile([1, H1], bf16)
            nc.sync.dma_start(out=w1c[:], in_=w1c_d[:])
            b1t = cp.tile([H1, 1], f32)
            nc.sync.dma_start(out=b1t[:], in_=b1_d[:])
            w2t = cp.tile([H1, H2], bf16)
            nc.sync.dma_start(out=w2t[:], in_=w2_d[:])
            b2t = cp.tile([H2, 1], f32)
            nc.sync.dma_start(out=b2t[:], in_=b2_d[:])

            h3 = cp.tile([3, EPC], bf16)
            foldg = cp.tile([128, 64], bf16)
            # w_sb[:, 504*f + e] = w_e[k, f]  (k on partitions, f-major cols:
            # the w-build copies stay contiguous; alpha lhsT reads stride EPC)
            w_sb = cp.tile([128, F * EPC], bf16)
            wv = w_sb[:].rearrange("p (f e) -> p e f", e=EPC)

            # ---------- per-edge hypernetwork MLP ----------
            with tc.tile_pool(name="pre", bufs=1) as pp, \
                 tc.tile_pool(name="preps", bufs=1, space="PSUM") as pps:
                hp1 = pps.tile([H1, EPC], f32, tag="hp1", space="PSUM")
                nc.tensor.matmul(out=hp1[:], lhsT=w1a[:], rhs=featA[:],
                                 start=True, stop=False)
                nc.tensor.matmul(out=hp1[:], lhsT=w1b[:], rhs=featB[:],
                                 start=False, stop=False)
                nc.tensor.matmul(out=hp1[:], lhsT=w1c[:], rhs=distR[:],
                                 start=False, stop=True)
                h1sb = pp.tile([H1, EPC], bf16, tag="h1sb")
                nc.scalar.activation(out=h1sb[:], in_=hp1[:], func=AF.Sigmoid,
                                     bias=b1t[:, 0:1])
                hp2 = pps.tile([H2, EPC], f32, tag="hp2", space="PSUM")
                nc.tensor.matmul(out=hp2[:], lhsT=w2t[:], rhs=h1sb[:],
                                 start=True, stop=True)
                nc.vector.memset(h3[:], 1.0)
                nc.scalar.activation(out=h3[0:2, :], in_=hp2[:], func=AF.Sigmoid,
                                     bias=b2t[:, 0:1])

                # foldg = foldm / sigmoid(gate): den fold absorbs the gate so
                # out = max(num,0) * (1/den') with den' = den/sig(gate)
                gt = pp.tile([1, 1], f32, tag="gt")
                nc.sync.dma_start(out=gt[:], in_=gate_d[:])
                gs = pp.tile([1, 1], f32, tag="gs")
                nc.scalar.activation(out=gs[:], in_=gt[:], func=AF.Sigmoid)
                gsr = pp.tile([1, 1], f32, tag="gsr")
                nc.vector.reciprocal(out=gsr[:], in_=gs[:])
                gp = pps.tile([128, 1], f32, tag="gp", space="PSUM")
                nc.tensor.matmul(out=gp[:], lhsT=ones1[:], rhs=gsr[:],
                                 start=True, stop=True)
                grb = pp.tile([128, 1], f32, tag="grb")
                nc.scalar.copy(out=grb[:], in_=gp[:])
                nc.vector.tensor_scalar_mul(foldg[:], foldm[:], grb[:, 0:1])

            # ---------- w build: one K=3 matmul per feature column f ----------
            with tc.tile_pool(name="wps", bufs=6, space="PSUM") as wps_p:
                for f in range(F):
                    wps = wps_p.tile([128, EPC], f32, tag="wps", space="PSUM")
                    nc.tensor.matmul(out=wps[:], lhsT=w3p[:, 128 * f:128 * (f + 1)],
                                     rhs=h3[:], start=True, stop=True)
                    dest = w_sb[:, EPC * f:EPC * (f + 1)]
                    if f % 2 == 0:
                        nc.vector.tensor_scalar_mul(dest, wps[:], 1.0)
                    else:
                        nc.scalar.copy(out=dest, in_=wps[:])

            # ---------- main per-node loop ----------
            outbuf = cp.tile([64, NPC * BT], bf16)
            NCHUNK = 3                          # nodes fetched per DMA
            with tc.tile_pool(name="rh", bufs=3) as rh_p, \
                 tc.tile_pool(name="lrl", bufs=4) as lrl_p, \
                 tc.tile_pool(name="ex", bufs=4) as ex_p, \
                 tc.tile_pool(name="pr", bufs=4) as pr_p, \
                 tc.tile_pool(name="fin", bufs=6) as fin_p, \
                 tc.tile_pool(name="aps", bufs=5, space="PSUM") as aps_p, \
                 tc.tile_pool(name="fps", bufs=3, space="PSUM") as fps_p:
                for j in range(NPC):
                    jc = j % NCHUNK
                    if jc == 0:
                        rspc = rh_p.tile([128, NCHUNK * 12 * BT], bf16, tag="rsp")
                        nc.sync.dma_start(
                            out=rspc[:],
                            in_=rsp_d[:, 12 * BT * j:12 * BT * (j + NCHUNK)])
                    rsp = rspc[:, 12 * BT * jc:12 * BT * (jc + 1)]
                    spk2 = rsp[:, 8 * BT:12 * BT]

                    lrl = lrl_p.tile([128, 4 * BT], bf16, tag="lrl")
                    aps = []
                    for hh in range(2):
                        eb = 8 * j + 4 * hh
                        c0 = 4 * hh * BT
                        ap = aps_p.tile([128, 2 * BT], f32, tag="ap", space="PSUM")
                        for q in range(4):
                            e = eb + q
                            rq = (q % 2) * 64              # psum row base
                            cq = (q // 2) * BT             # psum col base
                            nc.tensor.matmul(
                                out=ap[rq:rq + 64, cq:cq + BT],
                                lhsT=wv[:, e:e + 1, :],
                                rhs=rsp[:, c0 + q * BT:c0 + (q + 1) * BT],
                                start=True, stop=True)
                        aps.append(ap)

                        half = lrl[:, 2 * BT * hh:2 * BT * (hh + 1)]
                        if hh == 0:
                            # lrl = alpha - 0.99*min(alpha, 0) == leaky_relu
                            lt = lrl_p.tile([128, 2 * BT], f32, tag="lt")
                            nc.vector.tensor_scalar(
                                out=lt[:], in0=ap[:], scalar1=0.0, scalar2=-0.99,
                                op0=ALU.min, op1=ALU.mult)
                            nc.vector.tensor_add(out=half, in0=lt[:], in1=ap[:])
                        else:
                            nc.scalar.activation(out=half, in_=ap[:],
                                                 func=AF.Prelu, alpha=0.01)

                    ext = ex_p.tile([128, 4 * BT], bf16, tag="ext")
                    nc.scalar.activation(out=ext[:], in_=lrl[:], func=AF.Exp)
                    prt = pr_p.tile([128, 4 * BT], bf16, tag="prt")
                    nc.gpsimd.tensor_mul(out=prt[:], in0=ext[:], in1=spk2)

                    fold = fps_p.tile([64, 2 * BT], f32, tag="fold", space="PSUM")
                    for q in range(4):
                        nc.tensor.matmul(out=fold[:, 0:BT], lhsT=foldg[:],
                                         rhs=ext[:, q * BT:(q + 1) * BT],
                                         start=(q == 0), stop=(q == 3))
                    for q in range(4):
                        nc.tensor.matmul(out=fold[:, BT:2 * BT], lhsT=foldm[:],
                                         rhs=prt[:, q * BT:(q + 1) * BT],
                                         start=(q == 0), stop=(q == 3))

                    rden = fin_p.tile([64, BT], f32, tag="rden")
                    nc.vector.reciprocal_approx_fast(out=rden[:], in_=fold[:, 0:BT])
                    nc.vector.scalar_tensor_tensor(
                        out=outbuf[:, BT * j:BT * (j + 1)],
                        in0=fold[:, BT:2 * BT], scalar=0.0,
                        in1=rden[:], op0=ALU.max, op1=ALU.mult)
                nc.sync.dma_start(out=out_d[:], in_=outbuf[:])

    nc.compile()
    return nc


def _host_prep(state, feature, dist, src, dst, w1, b1, w2, b2, w3, b3, gate_weight):
    # sT[n, f, bt] in bf16 for pre-gathered src/dst state tiles
    sT = np.ascontiguousarray(
        state.transpose(2, 3, 0, 1).reshape(N, F, BT)).astype(ml_dtypes.bfloat16)
    sT_pad = np.zeros((NCORES * NPC, F, BT), ml_dtypes.bfloat16)
    sT_pad[:N] = sT

    # group edges by destination: edges_by[n, k] = id of n's k-th incoming edge
    order = np.argsort(dst, kind="stable")
    edges_by = order.reshape(N, KE)            # [N, KE]

    W3 = w3.reshape(2, 2 * F, F)
    # w3p[p, 128*f + k] = [W3_0 | W3_1 | B3][p][k, f]
    w3p = np.stack([W3[0], W3[1], b3.reshape(2 * F, F)], axis=0)  # [3, 128, 64]
    w3p = np.ascontiguousarray(w3p.transpose(0, 2, 1).reshape(3, F * 128)
                               ).astype(ml_dtypes.bfloat16)

    eye = np.eye(64, dtype=np.float32)
    base = {
        "w3p": w3p,
        "foldm": np.ascontiguousarray(np.concatenate([eye, eye], axis=0)
                                      ).astype(ml_dtypes.bfloat16),
        "w1ab": np.ascontiguousarray(w1[0:128]).astype(ml_dtypes.bfloat16),
        "w1c": np.ascontiguousarray(w1[128:129]).astype(ml_dtypes.bfloat16),
        "b1": np.ascontiguousarray(b1.reshape(H1, 1), np.float32),
        "w2": np.ascontiguousarray(w2).astype(ml_dtypes.bfloat16),
        "b2": np.ascontiguousarray(b2.reshape(H2, 1), np.float32),
        "gate": np.ascontiguousarray(gate_weight.reshape(1, 1), np.float32),
    }

    in_maps = []
    for c in range(NCORES):
        nodes = np.arange(c * NPC, (c + 1) * NPC)
        valid = nodes < N
        nodes_c = np.where(valid, nodes, 0)
        eids = edges_by[nodes_c]               # [NPC, KE] node-major edges
        src_c = np.where(valid[:, None], src[eids], 0).astype(np.int64)
        dist_c = np.where(valid[:, None], dist[eids, 0], 0.0).astype(np.float32)

        g = sT[src_c]                          # [NPC, KE, F, BT] bf16
        dn = sT_pad[nodes]                     # [NPC, F, BT] bf16

        # rsp: per node [128, 12*192].  Cols 0:8BT: col-block k rows 0:64 =
        # sT[src(e_k)], rows 64:128 = sT[dst node].  Cols 8BT:12BT: col-block
        # q rows 0:64 = sT[src(e_2q)], rows 64:128 = sT[src(e_2q+1)] (the
        # alpha-PSUM quadrant layout, for the exp*s_src product).
        gt = g.transpose(0, 2, 1, 3)           # [NPC, F, KE, BT]
        # [128 partitions, NPC nodes, 12 col-blocks, BT]
        rsp = np.empty((2, F, NPC, 12, BT), ml_dtypes.bfloat16)
        rsp[0, :, :, 0:8] = gt.transpose(1, 0, 2, 3)
        rsp[1, :, :, 0:8] = dn.transpose(1, 0, 2)[:, :, None, :]
        rsp[0, :, :, 8:12] = gt.transpose(1, 0, 2, 3)[:, :, 0::2]
        rsp[1, :, :, 8:12] = gt.transpose(1, 0, 2, 3)[:, :, 1::2]
        rsp = np.ascontiguousarray(rsp.reshape(128, NPC * 12 * BT))

        # featT: [128, EPC] = [feature[src].T ; feature[dst].T], node-major edges
        fsrc = np.where(valid[:, None, None], feature[src_c], 0.0)   # [NPC,KE,F]
        fdst = np.broadcast_to(
            np.where(valid[:, None], feature[nodes_c], 0.0)[:, None, :],
            (NPC, KE, F))
        featT = np.concatenate([
            fsrc.reshape(EPC, F).T, fdst.reshape(EPC, F).T], axis=0)

        m = dict(base)
        m.update({
            "rsp": rsp,
            "featT": np.ascontiguousarray(featT).astype(ml_dtypes.bfloat16),
            "distR": np.ascontiguousarray(dist_c.reshape(1, EPC)).astype(ml_dtypes.bfloat16),
        })
        in_maps.append(m)
    return in_maps


def kernel(state, feature, dist, w1, b1, w2, b2, w3, b3, gate_weight, src, dst):
    global LAST_RESULTS
    state = np.asarray(state, np.float32)
    if "nc" not in _cache:
        _cache["nc"] = _build_program()
    nc = _cache["nc"]

    in_maps = _host_prep(np.asarray(state), np.asarray(feature, np.float32),
                         np.asarray(dist, np.float32),
                         np.asarray(src), np.asarray(dst),
                         np.asarray(w1, np.float32), np.asarray(b1, np.float32),
                         np.asarray(w2, np.float32), np.asarray(b2, np.float32),
                         np.asarray(w3, np.float32), np.asarray(b3, np.float32),
                         np.asarray(gate_weight, np.float32))

    res = run_bass_kernel_spmd(nc, in_maps, core_ids=list(range(NCORES)), trace=TRACE)
    LAST_RESULTS = res

    # out_c: [64, NPC*BT] bf16 -> [F, NPC, B, T] -> [B, T, NPC, F]
    parts = [np.asarray(res.results[c]["out"]).astype(np.float32)
             .reshape(F, NPC, B, T).transpose(2, 3, 1, 0) for c in range(NCORES)]
    full = np.concatenate(parts, axis=2)[:, :, :N, :]
    return np.ascontiguousarray(full)


# revision 54
# speedup vs baseline: 1.2386x; 1.0976x over previous
"""MetaGAT message-passing kernel for Trainium2 (8 NeuronCores, Bass/Tile).

Strategy (node-sharded, fully local segment softmax, bf16 datapath):
  * dst has exactly K=8 incoming edges per node; edges are grouped by dst on
    the host and each core gets 63 nodes (500 padded to 504) with all their
    incoming edges.  Segment max/sum never crosses cores - no collectives.
  * The per-edge matmul inputs [s_src(e); s_dst(e)] (K=2F=128) are
    PRE-ASSEMBLED on the host into a bf16 tensor laid out exactly as the PE
    consumes them (one contiguous [128, 8*192] tile per node), so the device
    does plain sequential DMA - no indirect gathers, no SBUF reshuffles, and
    every alpha matmul is a single non-accumulating K=128 x M=64 x N=192
    bf16 op (start=stop=True; PE tile positions never mix inside a group).
  * A second host tensor holds the src states in the "quadrant" layout that
    matches the alpha PSUM tiles, for the exp()*s_src product.
  * The hypernetwork weights w_e = h0*W3_0 + h1*W3_1 + B3 for all 504 edges
    are built on the PE with 64 tiny K=3 matmuls (one per output column f):
    lhsT = [W3_0[:,f]; W3_1[:,f]; B3[:,f]] (3x128), rhs = [h0; h1; 1]
    (3x504).  This replaces ~260us of DVE broadcast work.
  * exp() without max-subtraction (|alpha| small for this data), products
    and pairwise sums in bf16, 128->64 fold + pair-sum via accumulating
    matmuls (uniform tile position), reciprocal via the fast DVE
    approximation with the sigmoid(gate) scale folded into the den fold
    weights, output written in [f, bt] layout (no PE transpose) and
    re-transposed on the host.
"""

import numpy as np
import ml_dtypes

import concourse.bacc as bacc
import concourse.bass as bass
import concourse.mybir as mybir
import concourse.tile as tile
from concourse.bass_utils import run_bass_kernel_spmd

N, E, KE, B, T, F = 500, 4000, 8, 16, 12, 64
BT = B * T                  # 192
NCORES = 8
NPC = 63                    # nodes per core (8*63 = 504 >= 500, tail padded)
EPC = KE * NPC              # 504 edges per core
H1, H2 = 16, 2
f32 = mybir.dt.float32
bf16 = mybir.dt.bfloat16
AF = mybir.ActivationFunctionType
ALU = mybir.AluOpType

TRACE = False               # set True (module-level) to profile; see LAST_RESULTS
LAST_RESULTS = None

_cache = {}


def _build_program():
    nc = bacc.Bacc("TRN2", target_bir_lowering=False)

    # per node, 12*BT cols: 0:8*BT alpha-matmul rhs blocks (src;dst per
    # edge), 8*BT:12*BT src states in alpha-PSUM quadrant layout.  Nodes are
    # side by side on columns so one DMA fetches several nodes.
    rsp_d = nc.dram_tensor("rsp", [128, NPC * 12 * BT], bf16, kind="ExternalInput")
    featT_d = nc.dram_tensor("featT", [128, EPC], bf16, kind="ExternalInput")
    distR_d = nc.dram_tensor("distR", [1, EPC], bf16, kind="ExternalInput")
    w1ab_d = nc.dram_tensor("w1ab", [128, H1], bf16, kind="ExternalInput")
    w1c_d = nc.dram_tensor("w1c", [1, H1], bf16, kind="ExternalInput")
    b1_d = nc.dram_tensor("b1", [H1, 1], f32, kind="ExternalInput")
    w2_d = nc.dram_tensor("w2", [H1, H2], bf16, kind="ExternalInput")
    b2_d = nc.dram_tensor("b2", [H2, 1], f32, kind="ExternalInput")
    w3p_d = nc.dram_tensor("w3p", [3, F * 128], bf16, kind="ExternalInput")
    gate_d = nc.dram_tensor("gate", [1, 1], f32, kind="ExternalInput")
    foldm_d = nc.dram_tensor("foldm", [128, 64], bf16, kind="ExternalInput")
    out_d = nc.dram_tensor("out", [64, NPC * BT], bf16, kind="ExternalOutput")

    with tile.TileContext(nc) as tc:
        with tc.tile_pool(name="const", bufs=1) as cp:
            foldm = cp.tile([128, 64], bf16)
            nc.sync.dma_start(out=foldm[:], in_=foldm_d[:])
            ones1 = cp.tile([1, 128], f32)
            nc.vector.memset(ones1[:], 1.0)
            w3p = cp.tile([3, F * 128], bf16)
            nc.sync.dma_start(out=w3p[:], in_=w3p_d[:])
            featA = cp.tile([64, EPC], bf16)
            nc.sync.dma_start(out=featA[:], in_=featT_d[0:64, :])
            featB = cp.tile([64, EPC], bf16)
            nc.sync.dma_start(out=featB[:], in_=featT_d[64:128, :])
            distR = cp.tile([1, EPC], bf16)
            nc.sync.dma_start(out=distR[:], in_=distR_d[:])
            w1a = cp.tile([64, H1], bf16)
            nc.sync.dma_start(out=w1a[:], in_=w1ab_d[0:64, :])
            w1b = cp.tile([64, H1], bf16)
            nc.sync.dma_start(out=w1b[:], in_=w1ab_d[64:128, :])
            w1c = cp.tile([1, H1], bf16)
            nc.sync.dma_start(out=w1c[:], in_=w1c_d[:])
            b1t = cp.tile([H1, 1], f32)
            nc.sync.dma_start(out=b1t[:], in_=b1_d[:])
            w2t = cp.tile([H1, H2], bf16)
            nc.sync.dma_start(out=w2t[:], in_=w2_d[:])
            b2t = cp.tile([H2, 1], f32)
            nc.sync.dma_start(out=b2t[:], in_=b2_d[:])

            h3 = cp.tile([3, EPC], bf16)
            foldg = cp.tile([128, 64], bf16)
            # w_sb[:, 504*f + e] = w_e[k, f]  (k on partitions, f-major cols:
            # the w-build copies stay contiguous; alpha lhsT reads stride EPC)
            w_sb = cp.tile([128, F * EPC], bf16)
            wv = w_sb[:].rearrange("p (f e) -> p e f", e=EPC)

            # ---------- per-edge hypernetwork MLP ----------
            with tc.tile_pool(name="pre", bufs=1) as pp, \
                 tc.tile_pool(name="preps", bufs=1, space="PSUM") as pps:
                hp1 = pps.tile([H1, EPC], f32, tag="hp1", space="PSUM")
                nc.tensor.matmul(out=hp1[:], lhsT=w1a[:], rhs=featA[:],
                                 start=True, stop=False)
                nc.tensor.matmul(out=hp1[:], lhsT=w1b[:], rhs=featB[:],
                                 start=False, stop=False)
                nc.tensor.matmul(out=hp1[:], lhsT=w1c[:], rhs=distR[:],
                                 start=False, stop=True)
                h1sb = pp.tile([H1, EPC], bf16, tag="h1sb")
                nc.scalar.activation(out=h1sb[:], in_=hp1[:], func=AF.Sigmoid,
                                     bias=b1t[:, 0:1])
                hp2 = pps.tile([H2, EPC], f32, tag="hp2", space="PSUM")
                nc.tensor.matmul(out=hp2[:], lhsT=w2t[:], rhs=h1sb[:],
                                 start=True, stop=True)
                nc.vector.memset(h3[:], 1.0)
                nc.scalar.activation(out=h3[0:2, :], in_=hp2[:], func=AF.Sigmoid,
                                     bias=b2t[:, 0:1])

                # foldg = foldm / sigmoid(gate): den fold absorbs the gate so
                # out = max(num,0) * (1/den') with den' = den/sig(gate)
                gt = pp.tile([1, 1], f32, tag="gt")
                nc.sync.dma_start(out=gt[:], in_=gate_d[:])
                gs = pp.tile([1, 1], f32, tag="gs")
                nc.scalar.activation(out=gs[:], in_=gt[:], func=AF.Sigmoid)
                gsr = pp.tile([1, 1], f32, tag="gsr")
                nc.vector.reciprocal(out=gsr[:], in_=gs[:])
                gp = pps.tile([128, 1], f32, tag="gp", space="PSUM")
                nc.tensor.matmul(out=gp[:], lhsT=ones1[:], rhs=gsr[:],
                                 start=True, stop=True)
                grb = pp.tile([128, 1], f32, tag="grb")
                nc.scalar.copy(out=grb[:], in_=gp[:])
                nc.vector.tensor_scalar_mul(foldg[:], foldm[:], grb[:, 0:1])

            # ---------- w build: one K=3 matmul per feature column f ----------
            with tc.tile_pool(name="wps", bufs=6, space="PSUM") as wps_p:
                for f in range(F):
                    wps = wps_p.tile([128, EPC], f32, tag="wps", space="PSUM")
                    nc.tensor.matmul(out=wps[:], lhsT=w3p[:, 128 * f:128 * (f + 1)],
                                     rhs=h3[:], start=True, stop=True)
                    dest = w_sb[:, EPC * f:EPC * (f + 1)]
                    if f % 2 == 0:
                        nc.vector.tensor_scalar_mul(dest, wps[:], 1.0)
                    else:
                        nc.scalar.copy(out=dest, in_=wps[:])

            # ---------- main per-node loop ----------
            outbuf = cp.tile([64, NPC * BT], bf16)
            NCHUNK = 3                          # nodes fetched per DMA
            with tc.tile_pool(name="rh", bufs=4) as rh_p, \
                 tc.tile_pool(name="lrl", bufs=4) as lrl_p, \
                 tc.tile_pool(name="ex", bufs=4) as ex_p, \
                 tc.tile_pool(name="pr", bufs=4) as pr_p, \
                 tc.tile_pool(name="fin", bufs=6) as fin_p, \
                 tc.tile_pool(name="aps", bufs=5, space="PSUM") as aps_p, \
                 tc.tile_pool(name="fps", bufs=3, space="PSUM") as fps_p:
                for j in range(NPC):
                    jc = j % NCHUNK
                    if jc == 0:
                        rspc = rh_p.tile([128, NCHUNK * 12 * BT], bf16, tag="rsp")
                        half = NCHUNK * 6 * BT
                        nc.sync.dma_start(
                            out=rspc[:, 0:half],
                            in_=rsp_d[:, 12 * BT * j:12 * BT * j + half])
                        nc.scalar.dma_start(
                            out=rspc[:, half:2 * half],
                            in_=rsp_d[:, 12 * BT * j + half:12 * BT * (j + NCHUNK)])
                    rsp = rspc[:, 12 * BT * jc:12 * BT * (jc + 1)]
                    spk2 = rsp[:, 8 * BT:12 * BT]

                    lrl = lrl_p.tile([128, 4 * BT], bf16, tag="lrl")
                    aps = []
                    for hh in range(2):
                        eb = 8 * j + 4 * hh
                        c0 = 4 * hh * BT
                        ap = aps_p.tile([128, 2 * BT], f32, tag="ap", space="PSUM")
                        for q in range(4):
                            e = eb + q
                            rq = (q % 2) * 64              # psum row base
                            cq = (q // 2) * BT             # psum col base
                            nc.tensor.matmul(
                                out=ap[rq:rq + 64, cq:cq + BT],
                                lhsT=wv[:, e:e + 1, :],
                                rhs=rsp[:, c0 + q * BT:c0 + (q + 1) * BT],
                                start=True, stop=True)
                        aps.append(ap)

                        half = lrl[:, 2 * BT * hh:2 * BT * (hh + 1)]
                        if hh == 0:
                            # lrl = alpha - 0.99*min(alpha, 0) == leaky_relu
                            lt = lrl_p.tile([128, 2 * BT], f32, tag="lt")
                            nc.vector.tensor_scalar(
                                out=lt[:], in0=ap[:], scalar1=0.0, scalar2=-0.99,
                                op0=ALU.min, op1=ALU.mult)
                            nc.vector.tensor_add(out=half, in0=lt[:], in1=ap[:])
                        else:
                            nc.scalar.activation(out=half, in_=ap[:],
                                                 func=AF.Prelu, alpha=0.01)

                    ext = ex_p.tile([128, 4 * BT], bf16, tag="ext")
                    nc.scalar.activation(out=ext[:], in_=lrl[:], func=AF.Exp)
                    prt = pr_p.tile([128, 4 * BT], bf16, tag="prt")
                    nc.gpsimd.tensor_mul(out=prt[:], in0=ext[:], in1=spk2)

                    fold = fps_p.tile([64, 2 * BT], f32, tag="fold", space="PSUM")
                    for q in range(4):
                        nc.tensor.matmul(out=fold[:, 0:BT], lhsT=foldg[:],
                                         rhs=ext[:, q * BT:(q + 1) * BT],
                                         start=(q == 0), stop=(q == 3))
                    for q in range(4):
                        nc.tensor.matmul(out=fold[:, BT:2 * BT], lhsT=foldm[:],
                                         rhs=prt[:, q * BT:(q + 1) * BT],
                                         start=(q == 0), stop=(q == 3))

                    rden = fin_p.tile([64, BT], f32, tag="rden")
                    nc.vector.reciprocal_approx_fast(out=rden[:], in_=fold[:, 0:BT])
                    nc.vector.scalar_tensor_tensor(
                        out=outbuf[:, BT * j:BT * (j + 1)],
                        in0=fold[:, BT:2 * BT], scalar=0.0,
                        in1=rden[:], op0=ALU.max, op1=ALU.mult)
                nc.sync.dma_start(out=out_d[:], in_=outbuf[:])

    nc.compile()
    return nc


def _host_prep(state, feature, dist, src, dst, w1, b1, w2, b2, w3, b3, gate_weight):
    # sT[n, f, bt] in bf16 for pre-gathered src/dst state tiles
    sT = np.ascontiguousarray(
        state.transpose(2, 3, 0, 1).reshape(N, F, BT)).astype(ml_dtypes.bfloat16)
    sT_pad = np.zeros((NCORES * NPC, F, BT), ml_dtypes.bfloat16)
    sT_pad[:N] = sT

    # group edges by destination: edges_by[n, k] = id of n's k-th incoming edge
    order = np.argsort(dst, kind="stable")
    edges_by = order.reshape(N, KE)            # [N, KE]

    W3 = w3.reshape(2, 2 * F, F)
    # w3p[p, 128*f + k] = [W3_0 | W3_1 | B3][p][k, f]
    w3p = np.stack([W3[0], W3[1], b3.reshape(2 * F, F)], axis=0)  # [3, 128, 64]
    w3p = np.ascontiguousarray(w3p.transpose(0, 2, 1).reshape(3, F * 128)
                               ).astype(ml_dtypes.bfloat16)

    eye = np.eye(64, dtype=np.float32)
    base = {
        "w3p": w3p,
        "foldm": np.ascontiguousarray(np.concatenate([eye, eye], axis=0)
                                      ).astype(ml_dtypes.bfloat16),
        "w1ab": np.ascontiguousarray(w1[0:128]).astype(ml_dtypes.bfloat16),
        "w1c": np.ascontiguousarray(w1[128:129]).astype(ml_dtypes.bfloat16),
        "b1": np.ascontiguousarray(b1.reshape(H1, 1), np.float32),
        "w2": np.ascontiguousarray(w2).astype(ml_dtypes.bfloat16),
        "b2": np.ascontiguousarray(b2.reshape(H2, 1), np.float32),
        "gate": np.ascontiguousarray(gate_weight.reshape(1, 1), np.float32),
    }

    in_maps = []
    for c in range(NCORES):
        nodes = np.arange(c * NPC, (c + 1) * NPC)
        valid = nodes < N
        nodes_c = np.where(valid, nodes, 0)
        eids = edges_by[nodes_c]               # [NPC, KE] node-major edges
        src_c = np.where(valid[:, None], src[eids], 0).astype(np.int64)
        dist_c = np.where(valid[:, None], dist[eids, 0], 0.0).astype(np.float32)

        g = sT[src_c]                          # [NPC, KE, F, BT] bf16
        dn = sT_pad[nodes]                     # [NPC, F, BT] bf16

        # rsp: per node [128, 12*192].  Cols 0:8BT: col-block k rows 0:64 =
        # sT[src(e_k)], rows 64:128 = sT[dst node].  Cols 8BT:12BT: col-block
        # q rows 0:64 = sT[src(e_2q)], rows 64:128 = sT[src(e_2q+1)] (the
        # alpha-PSUM quadrant layout, for the exp*s_src product).
        gt = g.transpose(0, 2, 1, 3)           # [NPC, F, KE, BT]
        # [128 partitions, NPC nodes, 12 col-blocks, BT]
        rsp = np.empty((2, F, NPC, 12, BT), ml_dtypes.bfloat16)
        rsp[0, :, :, 0:8] = gt.transpose(1, 0, 2, 3)
        rsp[1, :, :, 0:8] = dn.transpose(1, 0, 2)[:, :, None, :]
        rsp[0, :, :, 8:12] = gt.transpose(1, 0, 2, 3)[:, :, 0::2]
        rsp[1, :, :, 8:12] = gt.transpose(1, 0, 2, 3)[:, :, 1::2]
        rsp = np.ascontiguousarray(rsp.reshape(128, NPC * 12 * BT))

        # featT: [128, EPC] = [feature[src].T ; feature[dst].T], node-major edges
        fsrc = np.where(valid[:, None, None], feature[src_c], 0.0)   # [NPC,KE,F]
        fdst = np.broadcast_to(
            np.where(valid[:, None], feature[nodes_c], 0.0)[:, None, :],
            (NPC, KE, F))
        featT = np.concatenate([
            fsrc.reshape(EPC, F).T, fdst.reshape(EPC, F).T], axis=0)

        m = dict(base)
        m.update({
            "rsp": rsp,
            "featT": np.ascontiguousarray(featT).astype(ml_dtypes.bfloat16),
            "distR": np.ascontiguousarray(dist_c.reshape(1, EPC)).astype(ml_dtypes.bfloat16),
        })
        in_maps.append(m)
    return in_maps


def kernel(state, feature, dist, w1, b1, w2, b2, w3, b3, gate_weight, src, dst):
    global LAST_RESULTS
    state = np.asarray(state, np.float32)
    if "nc" not in _cache:
        _cache["nc"] = _build_program()
    nc = _cache["nc"]

    in_maps = _host_prep(np.asarray(state), np.asarray(feature, np.float32),
                         np.asarray(dist, np.float32),
                         np.asarray(src), np.asarray(dst),
                         np.asarray(w1, np.float32), np.asarray(b1, np.float32),
                         np.asarray(w2, np.float32), np.asarray(b2, np.float32),
                         np.asarray(w3, np.float32), np.asarray(b3, np.float32),
                         np.asarray(gate_weight, np.float32))

    res = run_bass_kernel_spmd(nc, in_maps, core_ids=list(range(NCORES)), trace=TRACE)
    LAST_RESULTS = res

    # out_c: [64, NPC*BT] bf16 -> [F, NPC, B, T] -> [B, T, NPC, F]
    parts = [np.asarray(res.results[c]["out"]).astype(np.float32)
             .reshape(F, NPC, B, T).transpose(2, 3, 1, 0) for c in range(NCORES)]
    full = np.concatenate(parts, axis=2)[:, :, :N, :]
    return np.ascontiguousarray(full)
